# revision 1
# baseline (speedup 1.0000x reference)
"""Single-head causal self-attention on 8 NeuronCores (data-parallel over batch).

Reference computation (per batch element b):
    Q = X @ Wq + bq; K = X @ Wk + bk; V = X @ Wv + bv        # [T, DK]
    S = Q @ K.T / sqrt(DK)  (causal masked)
    out = softmax(S) @ V                                      # [T, DK]

Device strategy (one batch element per core):
  - Host passes X.T [C, T] in bf16 so every DMA row is contiguous and
    half the bytes of fp32 (X DMA paces the projection phase).
  - Two full-width projection passes with packed stationaries:
      pass A: [Wv | Wk] -> psum rows 0:64 = V.T, rows 64:128 = K.T
      pass B: [Wq | Wq] -> Q.T duplicated in both partition halves
    Biases are added exactly during the PSUM->SBUF drain; pass-A drains
    on ScalarE (activation Identity + bias vec), pass-B on VectorE, both
    interleaved into the last k-tile's matmuls so the tail is one drain.
  - V.T is PE-transposed into natural [s, dk] tiles with a ones column
    appended; the ones column makes the output matmul also produce the
    softmax denominator l (row 64 of the output).
  - Scores are computed transposed: S.T[s, t] = K.T^T @ Q.T, so softmax's
    exp (ScalarE, scale=1/8 fused) and the P@V contraction (over s = the
    partition dim) need no large transposes. Causality = skip tiles below
    the diagonal + one upper-triangular 128x128 mask multiply per s-tile.
  - All matmul operands are bf16 (fp32 PSUM accumulation); fp32 tensors
    remain only in PSUM and the final output.
  - Device output per core: [65, T] = rows 0:64 unnormalized O.T, row 64 l.
    Host computes (O_unnorm / l).T in fp32.
"""

import sys

sys.path.insert(0, "/opt/trn_rl_repo")

import numpy as np
import ml_dtypes

B, T, C, DK = 8, 2048, 1024, 64
KT = C // 128          # 8 k-tiles in the contraction over C
NS = T // 128          # 16 s-tiles (key blocks)
NCHUNK = T // 512      # 4 output chunks of 512
SCALE = 1.0 / np.sqrt(DK)
BF16 = np.dtype(ml_dtypes.bfloat16)

_CACHE = {}


def _build():
    from concourse import bass, bacc, tile

    mybir = bass.mybir
    f32 = mybir.dt.float32
    bf16 = mybir.dt.bfloat16

    nc = bacc.Bacc(
        "TRN2", target_bir_lowering=False, debug=False, num_devices=B
    )

    xt_d = nc.dram_tensor("xt", [KT, 128, T], bf16, kind="ExternalInput")
    wvk_d = nc.dram_tensor("wvk", [128, KT * 128], bf16, kind="ExternalInput")
    wqq_d = nc.dram_tensor("wqq", [128, KT * 128], bf16, kind="ExternalInput")
    bvk_d = nc.dram_tensor("bvk", [128, 1], f32, kind="ExternalInput")
    bqq_d = nc.dram_tensor("bqq", [128, 1], f32, kind="ExternalInput")
    out_d = nc.dram_tensor("out", [65, T], bf16, kind="ExternalOutput")

    # one packed const block: cols 0:128 tri-mask, 128:192 ident (rows 0:64)
    cst_np = np.zeros((128, 192), dtype=BF16)
    cst_np[:, 0:128] = np.triu(np.ones((128, 128), dtype=np.float32)).astype(BF16)
    cst_np[0:64, 128:192] = np.eye(64, dtype=np.float32).astype(BF16)
    cst_d = nc.inline_tensor(cst_np, "cst")

    EXP = mybir.ActivationFunctionType.Exp

    with tile.TileContext(nc) as tc:
        with tc.tile_pool(name="const", bufs=1) as cpool, \
             tc.tile_pool(name="weights", bufs=1) as wpool, \
             tc.tile_pool(name="x", bufs=1) as xpool, \
             tc.tile_pool(name="acts", bufs=1) as apool:

            # small consts + weights first so their DMAs clear the queues
            # before the X stream
            cst = cpool.tile([128, 192], bf16)
            nc.gpsimd.dma_start(out=cst[:], in_=cst_d[:])
            tri = cst[:, 0:128]
            ident = cst[0:64, 128:192]
            bvk = cpool.tile([128, 1], f32)
            nc.gpsimd.dma_start(out=bvk[:], in_=bvk_d[:])
            bqq = cpool.tile([128, 1], f32)
            nc.gpsimd.dma_start(out=bqq[:], in_=bqq_d[:])

            wvk = wpool.tile([128, KT * 128], bf16)
            wqq = wpool.tile([128, KT * 128], bf16)
            nc.scalar.dma_start(out=wvk[:], in_=wvk_d[:])
            nc.sync.dma_start(out=wqq[:], in_=wqq_d[:])

            dma_engs = [nc.sync, nc.gpsimd, nc.scalar]
            xts = []
            for k in range(KT):
                xk = xpool.tile([128, T], bf16, tag=f"x{k}")
                dma_engs[k % 3].dma_start(out=xk[:], in_=xt_d[k])
                xts.append(xk)

            # persistent activations
            vk = apool.tile([128, T], bf16, tag="vk")    # V.T rows 0:64, K.T rows 64:128
            qq = apool.tile([128, T], bf16, tag="qq")    # Q.T in both halves
            v1 = apool.tile([128, NS * 65], bf16, tag="v1")  # [V_i | 1] stationaries
            osb = apool.tile([65, T], bf16, tag="osb")

            nc.gpsimd.memset(v1[:], 1.0)

            # ---------------- projections ----------------
            with tc.tile_pool(name="pproj", bufs=1, space="PSUM") as pproj:
                psA = pproj.tile([128, T], f32, tag="psA")
                psB = pproj.tile([128, T], f32, tag="psB")
                for k in range(KT):
                    last = k == KT - 1
                    for c in range(NCHUNK):
                        sl = slice(512 * c, 512 * (c + 1))
                        nc.tensor.matmul(
                            psA[:, sl],
                            wvk[:, 128 * k:128 * (k + 1)],
                            xts[k][:, sl],
                            start=(k == 0), stop=last,
                        )
                        if last:
                            # exact bias add during drain, on ScalarE
                            nc.scalar.add(vk[:, sl], psA[:, sl], bvk[:])
                    for c in range(NCHUNK):
                        sl = slice(512 * c, 512 * (c + 1))
                        nc.tensor.matmul(
                            psB[:, sl],
                            wqq[:, 128 * k:128 * (k + 1)],
                            xts[k][:, sl],
                            start=(k == 0), stop=last,
                        )
                        if last:
                            nc.vector.tensor_scalar_add(qq[:, sl], psB[:, sl], bqq[:])

            # ---------------- V transposes ----------------
            with tc.tile_pool(name="pv", bufs=2, space="PSUM") as pv:
                for i in range(NS):
                    vt = pv.tile([128, 64], bf16, tag="vt")
                    nc.tensor.transpose(
                        vt[:], vk[0:64, 128 * i:128 * (i + 1)], ident[:]
                    )
                    nc.vector.tensor_copy(v1[:, 65 * i:65 * i + 64], vt[:])

            # ---------------- attention ----------------
            with tc.tile_pool(name="po", bufs=1, space="PSUM") as po, \
                 tc.tile_pool(name="pst", bufs=2, space="PSUM") as pst, \
                 tc.tile_pool(name="et", bufs=3) as etpool:

                ops = [
                    po.tile([65, 512], f32, tag=f"o{j}", name=f"o{j}")
                    for j in range(NCHUNK)
                ]

                for i in range(NS):
                    ts = 128 * i
                    jmin = i // 4
                    et = etpool.tile([128, T], bf16, tag="et")
                    if ts > 512 * jmin:
                        nc.gpsimd.memset(et[:, 512 * jmin:ts], 0.0)
                    for tb in range(ts // 1024, 2):
                        st = pst.tile([128, 1024], f32, tag="st")
                        for cc in range(2):
                            t0 = 1024 * tb + 512 * cc
                            t1 = t0 + 512
                            if t1 <= ts:
                                continue
                            lo = max(t0, ts)  # exact 128-aligned diagonal start
                            nc.tensor.matmul(
                                st[:, lo - 1024 * tb:t1 - 1024 * tb],
                                vk[64:128, 128 * i:128 * (i + 1)],
                                qq[64:128, lo:t1],
                                start=True, stop=True,
                            )
                        off = max(0, ts - 1024 * tb)
                        nc.scalar.activation(
                            et[:, 1024 * tb + off:1024 * (tb + 1)],
                            st[:, off:1024],
                            EXP, scale=SCALE,
                        )
                    # causal mask on the diagonal 128-block
                    nc.vector.tensor_mul(
                        et[:, ts:ts + 128], et[:, ts:ts + 128], tri[:]
                    )
                    for j in range(jmin, NCHUNK):
                        nc.tensor.matmul(
                            ops[j][:],
                            v1[:, 65 * i:65 * i + 65],
                            et[:, 512 * j:512 * (j + 1)],
                            start=(i == 0), stop=(i == 4 * j + 3),
                        )
                    # drain any output chunk whose accumulation just finished
                    for j in range(jmin, NCHUNK):
                        if i == 4 * j + 3:
                            sl = slice(512 * j, 512 * (j + 1))
                            nc.vector.tensor_copy(osb[:, sl], ops[j][:])
                            nc.sync.dma_start(out=out_d[:, sl], in_=osb[:, sl])

    nc.compile()
    return nc


def _get_nc():
    if "nc" not in _CACHE:
        _CACHE["nc"] = _build()
    return _CACHE["nc"]


def make_in_maps(X, Wq, bq, Wk, bk, Wv, bv):
    X = np.asarray(X, dtype=np.float32)
    Wq = np.asarray(Wq, dtype=np.float32)
    Wk = np.asarray(Wk, dtype=np.float32)
    Wv = np.asarray(Wv, dtype=np.float32)
    bq = np.asarray(bq, dtype=np.float32)
    bk = np.asarray(bk, dtype=np.float32)
    bv = np.asarray(bv, dtype=np.float32)

    wvk = np.ascontiguousarray(
        np.concatenate([Wv, Wk], axis=1).reshape(KT, 128, 128)
        .transpose(1, 0, 2).reshape(128, KT * 128)
    ).astype(BF16)
    wqq = np.ascontiguousarray(
        np.concatenate([Wq, Wq], axis=1).reshape(KT, 128, 128)
        .transpose(1, 0, 2).reshape(128, KT * 128)
    ).astype(BF16)
    bvk = np.concatenate([bv, bk]).reshape(128, 1).astype(np.float32)
    bqq = np.concatenate([bq, bq]).reshape(128, 1).astype(np.float32)

    in_maps = []
    for b in range(B):
        xt = np.ascontiguousarray(X[b].T.astype(BF16)).reshape(KT, 128, T)
        in_maps.append(
            {"xt": xt, "wvk": wvk, "wqq": wqq, "bvk": bvk, "bqq": bqq}
        )
    return in_maps


def kernel(X, Wq, bq, Wk, bk, Wv, bv):
    from concourse.bass_utils import run_bass_kernel_spmd

    nc = _get_nc()
    in_maps = make_in_maps(X, Wq, bq, Wk, bk, Wv, bv)
    res = run_bass_kernel_spmd(nc, in_maps, list(range(B)))

    out = np.empty((B, T, DK), dtype=np.float32)
    for b in range(B):
        r = np.asarray(res.results[b]["out"], dtype=np.float32)
        out[b] = (r[:64] / r[64:65]).T
    return out



# revision 5
# speedup vs baseline: 1.0881x; 1.0881x over previous
"""Single-head causal self-attention on 8 NeuronCores (data-parallel over batch).

Reference computation (per batch element b):
    Q = X @ Wq + bq; K = X @ Wk + bk; V = X @ Wv + bv        # [T, DK]
    S = Q @ K.T / sqrt(DK)  (causal masked)
    out = softmax(S) @ V                                      # [T, DK]

V2 design (all bf16; fp8 was tested numerically and exceeds the error
budget in every variant):
  - X.T arrives in 4 column-chunks of 512 (bf16, ktile-major inside a
    chunk so each chunk is ONE contiguous 1MB DMA).  Chunks are DMA'd in
    DESCENDING order and projections run per-chunk as soon as each chunk
    lands, so the PE starts ~4us in instead of ~12us.
  - Projections: two packed 128-wide passes per chunk
      pass A: [Wv | Wk] -> psA rows 0:64 = V.T, rows 64:128 = K.T
      pass B: [Wq | Wq] -> Q.T duplicated (base-partition match for the
      scores matmul).  Drains: pass A on GpSimd (Pool), pass B on DVE;
    the Scalar (Act) engine is kept EXCLUSIVELY for exp.
  - Attention processes s-tile PAIRS (2x128 rows) in descending order,
    split into two t-halves (hi = t in [1024,2048), lo = [0,1024)) so
    only 2 output-PSUM banks are live at a time (8-bank budget:
    2 proj + 4 scores + 2 out).  Each (pair, 512-block) does 2 score
    matmuls into a [128,2,512] PSUM tile and ONE merged exp activation
    over both slots (20 activations total vs 24+, and Act does nothing
    else).
  - Causality: descending pairs skip below-diagonal blocks; the diagonal
    block gets an exact-range exp plus memset zero-fill + one
    upper-triangular mask multiply per slot.
  - P@V accumulates [65,512] f32 PSUM per t-block with a ones column in
    the V stationaries producing the softmax denominator (row 64).
  - Device output per core: [65, T]; host computes (O_unnorm / l).T.
"""

import sys

sys.path.insert(0, "/opt/trn_rl_repo")

import numpy as np
import ml_dtypes

B, T, C, DK = 8, 2048, 1024, 64
KT = C // 128            # 8 k-tiles in the contraction over C
NS = T // 128            # 16 s-tiles
NCHUNK = T // 512        # 4 chunks of 512
NP = NS // 2             # 8 s-tile pairs
SCALE = 1.0 / np.sqrt(DK)
BF16 = np.dtype(ml_dtypes.bfloat16)

_CACHE = {}


def _build():
    from concourse import bass, bacc, tile

    mybir = bass.mybir
    f32 = mybir.dt.float32
    bf16 = mybir.dt.bfloat16

    nc = bacc.Bacc(
        "TRN2", target_bir_lowering=False, debug=False, num_devices=B
    )

    # x chunks: [128, KT*512] each, ktile-major columns (one dense DMA)
    xc_d = [
        nc.dram_tensor(f"xc{c}", [128, KT * 512], bf16, kind="ExternalInput")
        for c in range(NCHUNK)
    ]
    wvk_d = nc.dram_tensor("wvk", [128, KT * 128], bf16, kind="ExternalInput")
    wqq_d = nc.dram_tensor("wqq", [128, KT * 128], bf16, kind="ExternalInput")
    bvk_d = nc.dram_tensor("bvk", [128, 1], f32, kind="ExternalInput")
    bqq_d = nc.dram_tensor("bqq", [128, 1], f32, kind="ExternalInput")
    out_d = nc.dram_tensor("out", [65, T], bf16, kind="ExternalOutput")

    # packed consts: cols 0:128 upper-tri mask, 128:192 identity (rows 0:64)
    cst_np = np.zeros((128, 192), dtype=BF16)
    cst_np[:, 0:128] = np.triu(np.ones((128, 128), dtype=np.float32)).astype(BF16)
    cst_np[0:64, 128:192] = np.eye(64, dtype=np.float32).astype(BF16)
    cst_d = nc.inline_tensor(cst_np, "cst")

    EXP = mybir.ActivationFunctionType.Exp

    def jmin(p):
        return (256 * p) // 512

    with tile.TileContext(nc) as tc:
        with tc.tile_pool(name="const", bufs=1) as cpool, \
             tc.tile_pool(name="weights", bufs=1) as wpool, \
             tc.tile_pool(name="x", bufs=1) as xpool, \
             tc.tile_pool(name="acts", bufs=1) as apool, \
             tc.tile_pool(name="et", bufs=3) as etpool, \
             tc.tile_pool(name="pp", bufs=1, space="PSUM") as pp, \
             tc.tile_pool(name="pst", bufs=2, space="PSUM") as pst, \
             tc.tile_pool(name="pops", bufs=2, space="PSUM") as pops:

            # small consts + weights on the gpsimd queue; X on sync queue
            cst = cpool.tile([128, 192], bf16)
            nc.gpsimd.dma_start(out=cst[:], in_=cst_d[:])
            tri = cst[:, 0:128]
            ident = cst[0:64, 128:192]
            bvk = cpool.tile([128, 1], f32)
            nc.gpsimd.dma_start(out=bvk[:], in_=bvk_d[:])
            bqq = cpool.tile([128, 1], f32)
            nc.gpsimd.dma_start(out=bqq[:], in_=bqq_d[:])
            wvk = wpool.tile([128, KT * 128], bf16)
            nc.gpsimd.dma_start(out=wvk[:], in_=wvk_d[:])
            wqq = wpool.tile([128, KT * 128], bf16)
            nc.gpsimd.dma_start(out=wqq[:], in_=wqq_d[:])

            # X chunks descending; first chunk split in two for faster start
            xs = [None] * NCHUNK
            for c in range(NCHUNK - 1, -1, -1):
                xk = xpool.tile([128, KT * 512], bf16, tag=f"x{c}")
                if c == NCHUNK - 1:
                    half = KT * 512 // 2
                    nc.sync.dma_start(out=xk[:, 0:half], in_=xc_d[c][:, 0:half])
                    nc.sync.dma_start(
                        out=xk[:, half:KT * 512], in_=xc_d[c][:, half:KT * 512]
                    )
                else:
                    nc.sync.dma_start(out=xk[:], in_=xc_d[c][:])
                xs[c] = xk

            # persistent activations
            vk = apool.tile([128, T], bf16, tag="vk")   # V.T rows 0:64, K.T rows 64:128
            qq = apool.tile([128, T], bf16, tag="qq")   # Q.T duplicated
            v1 = apool.tile([128, NS * 65], bf16, tag="v1")  # [V_i | 1] stationaries
            osb = apool.tile([65, T], bf16, tag="osb")

            nc.gpsimd.memset(v1[:], 1.0)

            # ---------------- projections (descending chunks) ----------------
            for c in range(NCHUNK - 1, -1, -1):
                sl = slice(512 * c, 512 * (c + 1))
                psA = pp.tile([128, 512], f32, tag="psA")
                for k in range(KT):
                    nc.tensor.matmul(
                        psA[:],
                        wvk[:, 128 * k:128 * (k + 1)],
                        xs[c][:, 512 * k:512 * (k + 1)],
                        start=(k == 0), stop=(k == KT - 1),
                    )
                nc.vector.tensor_scalar_add(vk[:, sl], psA[:], bvk[:])
                psB = pp.tile([128, 512], f32, tag="psB")
                for k in range(KT):
                    nc.tensor.matmul(
                        psB[:],
                        wqq[:, 128 * k:128 * (k + 1)],
                        xs[c][:, 512 * k:512 * (k + 1)],
                        start=(k == 0), stop=(k == KT - 1),
                    )
                nc.vector.tensor_scalar_add(qq[:, sl], psB[:], bqq[:])
                # V transposes for this chunk's 4 s-tiles
                for i in range(4 * c, 4 * c + 4):
                    vt = pp.tile([128, 64], bf16, tag="psA", name="vt")
                    nc.tensor.transpose(
                        vt[:], vk[0:64, 128 * i:128 * (i + 1)], ident[:]
                    )
                    nc.vector.tensor_copy(v1[:, 65 * i:65 * i + 64], vt[:])

            # ---------------- attention (two t-halves, descending pairs) ----
            for half_blocks, pairs in (
                ((2, 3), range(NP - 1, -1, -1)),
                ((0, 1), range(3, -1, -1)),
            ):
                hbase = 512 * half_blocks[0]
                otiles = {}
                for p in pairs:
                    i0, i1 = 2 * p, 2 * p + 1
                    ts0, ts1 = 128 * i0, 128 * i1
                    jm = jmin(p)
                    blocks = [b for b in half_blocks if b >= jm]
                    if not blocks:
                        continue
                    etp = etpool.tile([128, 2, 1024], bf16, tag="et")
                    for b in blocks:
                        s0 = max(ts0, 512 * b)
                        o0 = s0 - 512 * b
                        st = pst.tile([128, 2, 512], f32, tag="st")
                        nc.tensor.matmul(
                            st[:, 0, o0:512],
                            vk[64:128, 128 * i0:128 * (i0 + 1)],
                            qq[64:128, s0:512 * (b + 1)],
                            start=True, stop=True,
                        )
                        nc.tensor.matmul(
                            st[:, 1, o0:512],
                            vk[64:128, 128 * i1:128 * (i1 + 1)],
                            qq[64:128, s0:512 * (b + 1)],
                            start=True, stop=True,
                        )
                        e0 = s0 - hbase
                        e1 = 512 * (b + 1) - hbase
                        nc.scalar.activation(
                            etp[:, :, e0:e1], st[:, :, o0:512], EXP, scale=SCALE
                        )
                        if b == jm:
                            # zero below-diagonal lead-ins, tri-mask diagonal
                            if ts0 > 512 * b:
                                nc.gpsimd.memset(
                                    etp[:, 0, 512 * b - hbase:ts0 - hbase], 0.0
                                )
                            nc.gpsimd.memset(
                                etp[:, 1, 512 * b - hbase:ts1 - hbase], 0.0
                            )
                            nc.vector.tensor_mul(
                                etp[:, 0, ts0 - hbase:ts0 + 128 - hbase],
                                etp[:, 0, ts0 - hbase:ts0 + 128 - hbase],
                                tri[:],
                            )
                            nc.vector.tensor_mul(
                                etp[:, 1, ts1 - hbase:ts1 + 128 - hbase],
                                etp[:, 1, ts1 - hbase:ts1 + 128 - hbase],
                                tri[:],
                            )
                        # P @ [V|1] accumulation for this block
                        if b not in otiles:
                            otiles[b] = pops.tile(
                                [65, 512], f32, tag="o", name=f"o{b}"
                            )
                        pmax = min(2 * b + 1, NP - 1)
                        eb0 = 512 * b - hbase
                        nc.tensor.matmul(
                            otiles[b][:],
                            v1[:, 65 * i0:65 * i0 + 65],
                            etp[:, 0, eb0:eb0 + 512],
                            start=(p == pmax), stop=False,
                        )
                        nc.tensor.matmul(
                            otiles[b][:],
                            v1[:, 65 * i1:65 * i1 + 65],
                            etp[:, 1, eb0:eb0 + 512],
                            start=False, stop=(p == 0),
                        )
                # drain this half's outputs
                for b in half_blocks:
                    sl = slice(512 * b, 512 * (b + 1))
                    nc.vector.tensor_copy(osb[:, sl], otiles[b][:])
                    nc.sync.dma_start(out=out_d[:, sl], in_=osb[:, sl])

    nc.compile()
    return nc


def _get_nc():
    if "nc" not in _CACHE:
        _CACHE["nc"] = _build()
    return _CACHE["nc"]


def make_in_maps(X, Wq, bq, Wk, bk, Wv, bv):
    X = np.asarray(X, dtype=np.float32)
    Wq = np.asarray(Wq, dtype=np.float32)
    Wk = np.asarray(Wk, dtype=np.float32)
    Wv = np.asarray(Wv, dtype=np.float32)
    bq = np.asarray(bq, dtype=np.float32)
    bk = np.asarray(bk, dtype=np.float32)
    bv = np.asarray(bv, dtype=np.float32)

    wvk = np.ascontiguousarray(
        np.concatenate([Wv, Wk], axis=1).reshape(KT, 128, 128)
        .transpose(1, 0, 2).reshape(128, KT * 128)
    ).astype(BF16)
    wqq = np.ascontiguousarray(
        np.concatenate([Wq, Wq], axis=1).reshape(KT, 128, 128)
        .transpose(1, 0, 2).reshape(128, KT * 128)
    ).astype(BF16)
    bvk = np.concatenate([bv, bk]).reshape(128, 1).astype(np.float32)
    bqq = np.concatenate([bq, bq]).reshape(128, 1).astype(np.float32)

    in_maps = []
    for b in range(B):
        xt = X[b].T.astype(BF16)          # [C, T]
        m = {"wvk": wvk, "wqq": wqq, "bvk": bvk, "bqq": bqq}
        for c in range(NCHUNK):
            blk = xt[:, 512 * c:512 * (c + 1)]          # [1024, 512]
            m[f"xc{c}"] = np.ascontiguousarray(
                blk.reshape(KT, 128, 512).transpose(1, 0, 2).reshape(128, KT * 512)
            )
        in_maps.append(m)
    return in_maps


def kernel(X, Wq, bq, Wk, bk, Wv, bv):
    from concourse.bass_utils import run_bass_kernel_spmd

    nc = _get_nc()
    in_maps = make_in_maps(X, Wq, bq, Wk, bk, Wv, bv)
    res = run_bass_kernel_spmd(nc, in_maps, list(range(B)))

    out = np.empty((B, T, DK), dtype=np.float32)
    for b in range(B):
        r = np.asarray(res.results[b]["out"], dtype=np.float32)
        out[b] = (r[:64] / r[64:65]).T
    return out


# revision 7
# speedup vs baseline: 1.1788x; 1.0834x over previous
"""Single-head causal self-attention on 8 NeuronCores (data-parallel over batch).

Reference computation (per batch element b):
    Q = X @ Wq + bq; K = X @ Wk + bk; V = X @ Wv + bv        # [T, DK]
    S = Q @ K.T / sqrt(DK)  (causal masked)
    out = softmax(S) @ V                                      # [T, DK]

V2 design (all bf16; fp8 was tested numerically and exceeds the error
budget in every variant):
  - X.T arrives in 4 column-chunks of 512 (bf16, ktile-major inside a
    chunk so each chunk is ONE contiguous 1MB DMA).  Chunks are DMA'd in
    DESCENDING order and projections run per-chunk as soon as each chunk
    lands, so the PE starts ~4us in instead of ~12us.
  - Projections: two packed 128-wide passes per chunk
      pass A: [Wv | Wk] -> psA rows 0:64 = V.T, rows 64:128 = K.T
      pass B: [Wq | Wq] -> Q.T duplicated (base-partition match for the
      scores matmul).  Drains: pass A on GpSimd (Pool), pass B on DVE;
    the Scalar (Act) engine is kept EXCLUSIVELY for exp.
  - Attention processes s-tile PAIRS (2x128 rows) in descending order,
    split into two t-halves (hi = t in [1024,2048), lo = [0,1024)) so
    only 2 output-PSUM banks are live at a time (8-bank budget:
    2 proj + 4 scores + 2 out).  Each (pair, 512-block) does 2 score
    matmuls into a [128,2,512] PSUM tile and ONE merged exp activation
    over both slots (20 activations total vs 24+, and Act does nothing
    else).
  - Causality: descending pairs skip below-diagonal blocks; the diagonal
    block gets an exact-range exp plus memset zero-fill + one
    upper-triangular mask multiply per slot.
  - P@V accumulates [65,512] f32 PSUM per t-block with a ones column in
    the V stationaries producing the softmax denominator (row 64).
  - Device output per core: [65, T]; host computes (O_unnorm / l).T.
"""

import sys

sys.path.insert(0, "/opt/trn_rl_repo")

import numpy as np
import ml_dtypes

B, T, C, DK = 8, 2048, 1024, 64
KT = C // 128            # 8 k-tiles in the contraction over C
NS = T // 128            # 16 s-tiles
NCHUNK = T // 512        # 4 chunks of 512
NP = NS // 2             # 8 s-tile pairs
SCALE = 1.0 / np.sqrt(DK)
BF16 = np.dtype(ml_dtypes.bfloat16)

_CACHE = {}


def _build():
    from concourse import bass, bacc, tile

    mybir = bass.mybir
    f32 = mybir.dt.float32
    bf16 = mybir.dt.bfloat16

    nc = bacc.Bacc(
        "TRN2", target_bir_lowering=False, debug=False, num_devices=B
    )

    # x chunks: [128, KT*512] each, ktile-major columns (one dense DMA)
    xc_d = [
        nc.dram_tensor(f"xc{c}", [128, KT * 512], bf16, kind="ExternalInput")
        for c in range(NCHUNK)
    ]
    wvk_d = nc.dram_tensor("wvk", [128, KT * 128], bf16, kind="ExternalInput")
    wqq_d = nc.dram_tensor("wqq", [128, KT * 128], bf16, kind="ExternalInput")
    bvk_d = nc.dram_tensor("bvk", [128, 1], f32, kind="ExternalInput")
    bqq_d = nc.dram_tensor("bqq", [128, 1], f32, kind="ExternalInput")
    out_d = nc.dram_tensor("out", [65, T], bf16, kind="ExternalOutput")

    # packed consts: cols 0:128 upper-tri mask, 128:192 identity (rows 0:64)
    cst_np = np.zeros((128, 192), dtype=BF16)
    cst_np[:, 0:128] = np.triu(np.ones((128, 128), dtype=np.float32)).astype(BF16)
    cst_np[0:64, 128:192] = np.eye(64, dtype=np.float32).astype(BF16)
    cst_d = nc.inline_tensor(cst_np, "cst")

    EXP = mybir.ActivationFunctionType.Exp

    def jmin(p):
        return (256 * p) // 512

    with tile.TileContext(nc) as tc:
        with tc.tile_pool(name="const", bufs=1) as cpool, \
             tc.tile_pool(name="weights", bufs=1) as wpool, \
             tc.tile_pool(name="x", bufs=1) as xpool, \
             tc.tile_pool(name="acts", bufs=1) as apool, \
             tc.tile_pool(name="et", bufs=3) as etpool, \
             tc.tile_pool(name="pp", bufs=1, space="PSUM") as pp, \
             tc.tile_pool(name="pst", bufs=2, space="PSUM") as pst, \
             tc.tile_pool(name="pops", bufs=2, space="PSUM") as pops:

            # small consts + weights on the gpsimd queue; X on sync queue
            cst = cpool.tile([128, 192], bf16)
            nc.gpsimd.dma_start(out=cst[:], in_=cst_d[:])
            tri = cst[:, 0:128]
            ident = cst[0:64, 128:192]
            bvk = cpool.tile([128, 1], f32)
            nc.gpsimd.dma_start(out=bvk[:], in_=bvk_d[:])
            bqq = cpool.tile([128, 1], f32)
            nc.gpsimd.dma_start(out=bqq[:], in_=bqq_d[:])
            wvk = wpool.tile([128, KT * 128], bf16)
            nc.sync.dma_start(out=wvk[:], in_=wvk_d[:])
            wqq = wpool.tile([128, KT * 128], bf16)
            nc.sync.dma_start(out=wqq[:], in_=wqq_d[:])

            # X chunks descending; first chunk split in two for faster start
            xs = [None] * NCHUNK
            for c in range(NCHUNK - 1, -1, -1):
                xk = xpool.tile([128, KT * 512], bf16, tag=f"x{c}")
                if c == NCHUNK - 1:
                    half = KT * 512 // 2
                    nc.sync.dma_start(out=xk[:, 0:half], in_=xc_d[c][:, 0:half])
                    nc.sync.dma_start(
                        out=xk[:, half:KT * 512], in_=xc_d[c][:, half:KT * 512]
                    )
                else:
                    nc.sync.dma_start(out=xk[:], in_=xc_d[c][:])
                xs[c] = xk

            # persistent activations
            vk = apool.tile([128, T], bf16, tag="vk")   # V.T rows 0:64, K.T rows 64:128
            qq = apool.tile([128, T], bf16, tag="qq")   # Q.T duplicated
            v1 = apool.tile([128, NS * 65], bf16, tag="v1")  # [V_i | 1] stationaries
            osb = apool.tile([65, T], bf16, tag="osb")

            nc.gpsimd.memset(v1[:], 1.0)

            # PE pre-warm: HAM un-throttles (1.2 -> 2.4 GHz) only after
            # ~3.4us of sustained PE activity.  Run dummy matmuls on a
            # zeroed scratch tile while the weight/X DMAs are in flight so
            # real matmuls start warm.
            warm_in = cpool.tile([128, 512], bf16, name="warm_in")
            nc.gpsimd.memset(warm_in[:], 0.0)
            for w in range(12):
                wps = pst.tile([128, 512], f32, tag="st", name="warm_ps")
                nc.tensor.matmul(
                    wps[:], warm_in[:, 0:128], warm_in[:],
                    start=True, stop=True,
                )

            # ---------------- projections (descending chunks) ----------------
            for c in range(NCHUNK - 1, -1, -1):
                sl = slice(512 * c, 512 * (c + 1))
                psA = pp.tile([128, 512], f32, tag="psA")
                for k in range(KT):
                    nc.tensor.matmul(
                        psA[:],
                        wvk[:, 128 * k:128 * (k + 1)],
                        xs[c][:, 512 * k:512 * (k + 1)],
                        start=(k == 0), stop=(k == KT - 1),
                    )
                nc.vector.tensor_scalar_add(vk[:, sl], psA[:], bvk[:])
                psB = pp.tile([128, 512], f32, tag="psB")
                for k in range(KT):
                    nc.tensor.matmul(
                        psB[:],
                        wqq[:, 128 * k:128 * (k + 1)],
                        xs[c][:, 512 * k:512 * (k + 1)],
                        start=(k == 0), stop=(k == KT - 1),
                    )
                nc.vector.tensor_scalar_add(qq[:, sl], psB[:], bqq[:])
                # V transposes for this chunk's 4 s-tiles
                for i in range(4 * c, 4 * c + 4):
                    vt = pp.tile([128, 64], bf16, tag="psA", name="vt")
                    nc.tensor.transpose(
                        vt[:], vk[0:64, 128 * i:128 * (i + 1)], ident[:]
                    )
                    nc.vector.tensor_copy(v1[:, 65 * i:65 * i + 64], vt[:])

            # ---------------- attention (two t-halves, descending pairs) ----
            for half_blocks, pairs in (
                ((2, 3), list(range(NP - 1, -1, -1))),
                ((0, 1), [0, 1, 2, 3]),
            ):
                hbase = 512 * half_blocks[0]
                otiles = {}
                for p in pairs:
                    i0, i1 = 2 * p, 2 * p + 1
                    ts0, ts1 = 128 * i0, 128 * i1
                    jm = jmin(p)
                    blocks = [b for b in half_blocks if b >= jm]
                    if not blocks:
                        continue
                    etp = etpool.tile([128, 2, 1024], bf16, tag="et")
                    for b in blocks:
                        s0 = max(ts0, 512 * b)
                        o0 = s0 - 512 * b
                        st = pst.tile([128, 2, 512], f32, tag="st")
                        nc.tensor.matmul(
                            st[:, 0, o0:512],
                            vk[64:128, 128 * i0:128 * (i0 + 1)],
                            qq[64:128, s0:512 * (b + 1)],
                            start=True, stop=True,
                        )
                        nc.tensor.matmul(
                            st[:, 1, o0:512],
                            vk[64:128, 128 * i1:128 * (i1 + 1)],
                            qq[64:128, s0:512 * (b + 1)],
                            start=True, stop=True,
                        )
                        e0 = s0 - hbase
                        e1 = 512 * (b + 1) - hbase
                        nc.scalar.activation(
                            etp[:, :, e0:e1], st[:, :, o0:512], EXP, scale=SCALE
                        )
                        if b == jm:
                            # zero below-diagonal lead-ins, tri-mask diagonal
                            if ts0 > 512 * b:
                                nc.gpsimd.memset(
                                    etp[:, 0, 512 * b - hbase:ts0 - hbase], 0.0
                                )
                            nc.gpsimd.memset(
                                etp[:, 1, 512 * b - hbase:ts1 - hbase], 0.0
                            )
                            nc.vector.tensor_mul(
                                etp[:, 0, ts0 - hbase:ts0 + 128 - hbase],
                                etp[:, 0, ts0 - hbase:ts0 + 128 - hbase],
                                tri[:],
                            )
                            nc.vector.tensor_mul(
                                etp[:, 1, ts1 - hbase:ts1 + 128 - hbase],
                                etp[:, 1, ts1 - hbase:ts1 + 128 - hbase],
                                tri[:],
                            )
                        # P @ [V|1] accumulation for this block
                        if b not in otiles:
                            otiles[b] = pops.tile(
                                [65, 512], f32, tag="o", name=f"o{b}"
                            )
                        contrib = [q for q in pairs
                                   if b in [x for x in half_blocks if x >= jmin(q)]]
                        eb0 = 512 * b - hbase
                        nc.tensor.matmul(
                            otiles[b][:],
                            v1[:, 65 * i0:65 * i0 + 65],
                            etp[:, 0, eb0:eb0 + 512],
                            start=(p == contrib[0]), stop=False,
                        )
                        nc.tensor.matmul(
                            otiles[b][:],
                            v1[:, 65 * i1:65 * i1 + 65],
                            etp[:, 1, eb0:eb0 + 512],
                            start=False, stop=(p == contrib[-1]),
                        )
                    # drain any block whose accumulation just finished
                    for b in blocks:
                        contrib = [q for q in pairs
                                   if b in [x for x in half_blocks if x >= jmin(q)]]
                        if p == contrib[-1]:
                            sl = slice(512 * b, 512 * (b + 1))
                            nc.vector.tensor_copy(osb[:, sl], otiles[b][:])
                            nc.sync.dma_start(out=out_d[:, sl], in_=osb[:, sl])

    nc.compile()
    return nc


def _get_nc():
    if "nc" not in _CACHE:
        _CACHE["nc"] = _build()
    return _CACHE["nc"]


def make_in_maps(X, Wq, bq, Wk, bk, Wv, bv):
    X = np.asarray(X, dtype=np.float32)
    Wq = np.asarray(Wq, dtype=np.float32)
    Wk = np.asarray(Wk, dtype=np.float32)
    Wv = np.asarray(Wv, dtype=np.float32)
    bq = np.asarray(bq, dtype=np.float32)
    bk = np.asarray(bk, dtype=np.float32)
    bv = np.asarray(bv, dtype=np.float32)

    wvk = np.ascontiguousarray(
        np.concatenate([Wv, Wk], axis=1).reshape(KT, 128, 128)
        .transpose(1, 0, 2).reshape(128, KT * 128)
    ).astype(BF16)
    wqq = np.ascontiguousarray(
        np.concatenate([Wq, Wq], axis=1).reshape(KT, 128, 128)
        .transpose(1, 0, 2).reshape(128, KT * 128)
    ).astype(BF16)
    bvk = np.concatenate([bv, bk]).reshape(128, 1).astype(np.float32)
    bqq = np.concatenate([bq, bq]).reshape(128, 1).astype(np.float32)

    in_maps = []
    for b in range(B):
        xt = X[b].T.astype(BF16)          # [C, T]
        m = {"wvk": wvk, "wqq": wqq, "bvk": bvk, "bqq": bqq}
        for c in range(NCHUNK):
            blk = xt[:, 512 * c:512 * (c + 1)]          # [1024, 512]
            m[f"xc{c}"] = np.ascontiguousarray(
                blk.reshape(KT, 128, 512).transpose(1, 0, 2).reshape(128, KT * 512)
            )
        in_maps.append(m)
    return in_maps


def kernel(X, Wq, bq, Wk, bk, Wv, bv):
    from concourse.bass_utils import run_bass_kernel_spmd

    nc = _get_nc()
    in_maps = make_in_maps(X, Wq, bq, Wk, bk, Wv, bv)
    res = run_bass_kernel_spmd(nc, in_maps, list(range(B)))

    out = np.empty((B, T, DK), dtype=np.float32)
    for b in range(B):
        r = np.asarray(res.results[b]["out"], dtype=np.float32)
        out[b] = (r[:64] / r[64:65]).T
    return out
